# revision 23
# baseline (speedup 1.0000x reference)
"""Trainium2 Bass kernel for nn_MindPalaceRouter.

Computation (reference):
    ctx  = mean_T(x) @ Wc.T + bc                      [B, d]
    warp = (ctx @ Ww.T + bw).reshape(B, n, n) * 0.1
    adj  = softmax(adjacency + warp, axis=-1)
    raw  = ctx @ summaries.T                          [B, n]
    gate = sigmoid((ctx @ Wg.T + bg + adj @ raw) / 2) [B, n]

Strategy: data-parallel over B across 8 cores (4 samples/core); weights
replicated (host pre-transposed so matmul contraction dims land on SBUF
partitions; weights cast bf16, x stays f32). The x reduction over T is done
on the tensor engine (x tiles as stationary operand, ones as moving) and
accumulates mean_xT directly in d-on-partitions layout. Biases and the
adjacency offset are folded into the PSUM accumulations as rank-1 matmuls
(ones/tens column outer-product rows) so no partition-broadcast is needed.
Softmax skips max-subtraction (logits bounded ~|2.5|) and fuses the 0.1
scale into the Exp activation.
"""

import sys

if "/opt/trn_rl_repo" not in sys.path:
    sys.path.insert(0, "/opt/trn_rl_repo")

import numpy as np

N_CORES = 8
B, T, D, NR = 32, 2048, 1024, 64
NN = NR * NR  # 4096
BSH = B // N_CORES  # 4 samples per core
TCH = T // 128  # 16 T-chunks
DCH = D // 128  # 8 d-chunks

_cache = {}


def _build_nc():
    import concourse.bass as bass
    import concourse.tile as tile
    from concourse import bacc, mybir
    from concourse.masks import make_identity

    f32 = mybir.dt.float32
    bf16 = mybir.dt.bfloat16
    f8 = mybir.dt.float8e4
    AF = mybir.ActivationFunctionType

    nc = bacc.Bacc(
        "TRN2",
        target_bir_lowering=False,
        debug=False,
        enable_asserts=True,
        num_devices=N_CORES,
    )

    xs_d = nc.dram_tensor("xs", [BSH, T, D], bf16, kind="ExternalInput")
    wct_d = nc.dram_tensor("WcT", [D, D], bf16, kind="ExternalInput")
    bc_d = nc.dram_tensor("bc", [1, D], f32, kind="ExternalInput")
    wgt_d = nc.dram_tensor("WgT", [128, DCH * NR], bf16, kind="ExternalInput")
    bg_d = nc.dram_tensor("bg", [1, NR], f32, kind="ExternalInput")
    wwt_d = nc.dram_tensor("WwT", [D, NN], f8, kind="ExternalInput")
    bw_d = nc.dram_tensor("bw", [1, NN], f32, kind="ExternalInput")
    adj_d = nc.dram_tensor("adjf", [1, NN], f32, kind="ExternalInput")
    smt_d = nc.dram_tensor("smT", [128, DCH * NR], bf16, kind="ExternalInput")
    out_d = nc.dram_tensor("gates", [BSH, NR], f32, kind="ExternalOutput")

    with tile.TileContext(nc) as tc:
        with (
            tc.tile_pool(name="const", bufs=1) as constp,
            tc.tile_pool(name="xin", bufs=8) as xin,
            tc.tile_pool(name="wc_hold", bufs=1) as wc_hold,
            tc.tile_pool(name="ww_hold", bufs=1) as ww_hold,
            tc.tile_pool(name="mid", bufs=1) as mid,
        ):
            # --- constants ---
            ones128 = constp.tile([128, 1], bf16)
            nc.gpsimd.memset(ones128[:], 1.0)
            ones4 = constp.tile([1, BSH], f32)
            nc.gpsimd.memset(ones4[:], 1.0)
            ones4b = constp.tile([1, BSH], bf16)
            nc.gpsimd.memset(ones4b[:], 1.0)
            ones1b = constp.tile([1, 1], bf16)
            nc.gpsimd.memset(ones1b[:], 1.0)
            ident4 = constp.tile([BSH, BSH], bf16)
            make_identity(nc, ident4[:])
            # Preload the Exp ACT table so no ACT_TABLE_LOAD lands on the
            # critical path later (sigmoid is computed via Exp+reciprocal).
            scratch_act = constp.tile([1, BSH], f32)
            nc.scalar.activation(scratch_act[:], ones4[:], AF.Exp)

            bc_row = constp.tile([1, D], f32)
            bg_row = constp.tile([1, NR], f32)
            bw_row = constp.tile([1, NN], f32)
            adj_row = constp.tile([1, NN], f32)
            comb_row = constp.tile([1, NN], f32)
            comb_bf = constp.tile([1, NN], bf16)
            smt_t = constp.tile([128, DCH * NR], bf16)
            wgt_t = constp.tile([128, DCH * NR], bf16)
            wc_tiles = [
                wc_hold.tile([128, D], bf16, tag=f"wc{j}", name=f"wc{j}")
                for j in range(DCH)
            ]
            ww_tiles = [
                ww_hold.tile(
                    [128, NN // 2], f8, tag=f"ww{h}_{j}", name=f"ww{h}_{j}"
                )
                for h in range(2)
                for j in range(DCH)
            ]

            # Weight/const DMAs ride the SCALAR engine queue (x owns the
            # sync and gpsimd queues), ordered by first use: WcT/consts,
            # then WwT.  The final WwT half-1 tiles are instead issued on
            # the sync/gpsimd queues AFTER all x tiles, so the DMA trail
            # runs multi-queue instead of single-queue.
            def issue_weight_dmas():
                for j in range(DCH):
                    nc.scalar.dma_start(
                        wc_tiles[j][:], wct_d[j * 128 : (j + 1) * 128, :]
                    )
                nc.scalar.dma_start(bc_row[:], bc_d[:])
                nc.scalar.dma_start(smt_t[:], smt_d[:])
                nc.scalar.dma_start(wgt_t[:], wgt_d[:])
                nc.scalar.dma_start(bg_row[:], bg_d[:])
                nc.scalar.dma_start(bw_row[:], bw_d[:])
                nc.scalar.dma_start(adj_row[:], adj_d[:])
                for j in range(DCH):
                    nc.scalar.dma_start(
                        ww_tiles[j][:],
                        wwt_d[j * 128 : (j + 1) * 128, 0 : NN // 2],
                    )

            def issue_ww_trail(eng, j):
                nc_eng = {"sync": nc.sync, "gpsimd": nc.gpsimd}[eng]
                nc_eng.dma_start(
                    ww_tiles[DCH + j][:],
                    wwt_d[j * 128 : (j + 1) * 128, NN // 2 : NN],
                )

            # --- phase A: sum_T(x) -> mean_x [BSH, D] (bf16 rows) ---
            # ones128 is the stationary operand (loaded once); x tiles
            # stream through the PE as N=512 matmuls accumulating row
            # sums in a 2-bank PSUM strip per sample.
            mean_xT = mid.tile([128, DCH * BSH], bf16)
            issue_weight_dmas()
            # comb = 10*adjacency + bw on DVE while it is idle; bf16 copy
            # feeds cheap rank-1 matmuls in the warp accumulation groups.
            nc.vector.tensor_scalar(
                comb_row[:], adj_row[:], 10.0, None, mybir.AluOpType.mult
            )
            nc.vector.tensor_add(comb_row[:], comb_row[:], bw_row[:])
            nc.vector.tensor_copy(comb_bf[:], comb_row[:])
            with tc.tile_pool(name="pmean", bufs=1, space="PSUM") as pmean_p:
                NHALF = TCH // 2
                for b in range(BSH):
                    pmean = pmean_p.tile([128, DCH * 512], f32, tag="pmean")
                    for it2 in range(NHALF):
                        xt = xin.tile([128, 2 * D], bf16, name="xt", tag="xt")
                        eng = nc.sync if (it2 % 2 == 0) else nc.gpsimd
                        eng.dma_start(
                            xt[:].rearrange("p (s d) -> p s d", d=D),
                            xs_d[
                                b, it2 * 256 : (it2 + 1) * 256, :
                            ].rearrange("(s p) d -> p s d", s=2),
                        )
                        for s in range(2):
                            for j in range(DCH):
                                nc.tensor.matmul(
                                    pmean[:, j * 512 : j * 512 + 1],
                                    xt[
                                        :,
                                        s * D + j * 128 : s * D + (j + 1) * 128,
                                    ],
                                    ones128[:],
                                    start=(it2 == 0 and s == 0),
                                    stop=(it2 == NHALF - 1 and s == 1),
                                )
                    # trailing WwT half-1 tiles ride the freed x queues
                    if b == BSH - 1:
                        for j in range(DCH):
                            issue_ww_trail("sync" if j % 2 == 0 else "gpsimd", j)
                    srcv = pmean[:].rearrange("p (j c) -> p j c", c=512)[:, :, 0:1]
                    dstv = mean_xT[:].rearrange("p (j b) -> p j b", b=BSH)[
                        :, :, b : b + 1
                    ]
                    nc.vector.tensor_scalar_mul(dstv, srcv, 1.0 / T)

            # --- phase B: ctx = mean @ WcT + bc, half-major so the first
            # half's copy/transposes overlap the second half's matmuls ---
            ctx_s = mid.tile([BSH, D], bf16)
            with (
                tc.tile_pool(name="pctx", bufs=1, space="PSUM") as pctx_p,
                tc.tile_pool(name="pt", bufs=1, space="PSUM") as pt_p,
            ):
                pctx = pctx_p.tile([BSH, D], f32)
                for h in range(2):
                    for j in range(DCH):
                        nc.tensor.matmul(
                            pctx[:, h * 512 : (h + 1) * 512],
                            mean_xT[:, j * BSH : (j + 1) * BSH],
                            wc_tiles[j][:, h * 512 : (h + 1) * 512],
                            start=(j == 0),
                            stop=False,
                        )
                    nc.tensor.matmul(
                        pctx[:, h * 512 : (h + 1) * 512],
                        ones4[:],
                        bc_row[:, h * 512 : (h + 1) * 512],
                        start=False,
                        stop=True,
                    )
                    nc.scalar.copy(
                        ctx_s[:, h * 512 : (h + 1) * 512],
                        pctx[:, h * 512 : (h + 1) * 512],
                    )

                # transpose ctx -> ctxT [128, DCH*BSH]
                ptr = pt_p.tile([128, DCH * BSH], bf16)
                for j in range(DCH):
                    nc.tensor.transpose(
                        ptr[:, j * BSH : (j + 1) * BSH],
                        ctx_s[:, j * 128 : (j + 1) * 128],
                        ident4[:],
                    )
                ctxT = mid.tile([128, DCH * BSH], bf16)
                nc.vector.tensor_copy(ctxT[:], ptr[:])

            # --- raw = ctx @ smT ; gate_part = ctx @ WgT + bg ---
            raw_s = mid.tile([BSH, NR], f32)
            g_s = mid.tile([BSH, NR], f32)
            with tc.tile_pool(name="psm", bufs=1, space="PSUM") as psm_p:
                praw = psm_p.tile([BSH, NR], f32, tag="praw")
                pg = psm_p.tile([BSH, NR], f32, tag="pg")
                for j in range(DCH):
                    nc.tensor.matmul(
                        praw[:],
                        ctxT[:, j * BSH : (j + 1) * BSH],
                        smt_t[:, j * NR : (j + 1) * NR],
                        start=(j == 0),
                        stop=(j == DCH - 1),
                    )
                for j in range(DCH):
                    nc.tensor.matmul(
                        pg[:],
                        ctxT[:, j * BSH : (j + 1) * BSH],
                        wgt_t[:, j * NR : (j + 1) * NR],
                        start=(j == 0),
                        stop=False,
                    )
                nc.tensor.matmul(pg[:], ones4[:], bg_row[:], start=False, stop=True)
                nc.scalar.copy(raw_s[:], praw[:])
                nc.scalar.copy(g_s[:], pg[:])

            # --- warp (+adjacency/bw via one bf16 rank-1) in single-bank
            # eighths: matmuls, exp and the softmax-tail DVE chain pipeline
            # across eighths, so only the last eighth's chain is serial ---
            adj_exp = mid.tile([BSH, NN], bf16)
            ssum = mid.tile([BSH, NR], f32)
            qsum = mid.tile([BSH, NR], f32)
            qprod = mid.tile([BSH, 512], bf16)
            raw_bf = mid.tile([BSH, NR], bf16)
            nc.vector.tensor_copy(raw_bf[:], raw_s[:])
            NE = 512  # columns per eighth = 8 m-groups of 64
            GE = NE // NR  # m-groups per eighth
            raw_b = raw_bf[:].unsqueeze(1).to_broadcast([BSH, GE, NR])
            with tc.tile_pool(name="pw", bufs=3, space="PSUM") as pw_p:
                for e in range(8):
                    pw = pw_p.tile([BSH, NE], f32, name="pw", tag="pw")
                    half, c2 = e // 4, e % 4
                    nc.tensor.matmul(
                        pw[:],
                        ones4b[:],
                        comb_bf[:, e * NE : (e + 1) * NE],
                        start=True,
                        stop=False,
                    )
                    for j in range(DCH):
                        wt = ww_tiles[half * DCH + j]
                        nc.tensor.matmul(
                            pw[:],
                            ctxT[:, j * BSH : (j + 1) * BSH],
                            wt[:, c2 * NE : (c2 + 1) * NE],
                            start=False,
                            stop=(j == DCH - 1),
                        )
                    ae = adj_exp[:, e * NE : (e + 1) * NE]
                    nc.scalar.activation(ae, pw[:], AF.Exp, bias=0.0, scale=0.1)
                    aeg = ae.rearrange("p (m n) -> p m n", n=NR)
                    nc.vector.reduce_sum(
                        ssum[:, e * GE : (e + 1) * GE],
                        aeg,
                        axis=mybir.AxisListType.X,
                    )
                    qp = qprod[:].rearrange("p (m n) -> p m n", n=NR)
                    nc.vector.tensor_mul(qp, aeg, raw_b)
                    nc.vector.reduce_sum(
                        qsum[:, e * GE : (e + 1) * GE],
                        qp,
                        axis=mybir.AxisListType.X,
                    )

            # --- finals: gates = 1 / (1 + exp(-(g + qsum/ssum) / 2)) ---
            rinv = mid.tile([BSH, NR], f32)
            nc.vector.reciprocal(rinv[:], ssum[:])
            extra = mid.tile([BSH, NR], f32)
            nc.vector.tensor_mul(extra[:], qsum[:], rinv[:])
            logits = mid.tile([BSH, NR], f32)
            nc.vector.tensor_add(logits[:], g_s[:], extra[:])
            en = mid.tile([BSH, NR], f32)
            nc.scalar.activation(en[:], logits[:], AF.Exp, bias=0.0, scale=-0.5)
            ep1 = mid.tile([BSH, NR], f32)
            nc.vector.tensor_scalar_add(ep1[:], en[:], 1.0)
            gates_s = mid.tile([BSH, NR], f32)
            nc.vector.reciprocal(gates_s[:], ep1[:])
            nc.sync.dma_start(out_d[:], gates_s[:])

    nc.compile()
    return nc


def _get_nc():
    if "nc" not in _cache:
        _cache["nc"] = _build_nc()
    return _cache["nc"]


def _make_in_maps(x, summaries, Wc, bc, Wg, bg, Ww, bw, adjacency):
    import ml_dtypes

    bf16 = ml_dtypes.bfloat16
    f32 = np.float32

    x = np.ascontiguousarray(np.asarray(x, dtype=f32).astype(bf16))
    WcT = np.ascontiguousarray(np.asarray(Wc, dtype=f32).T.astype(bf16))
    f8np = __import__('ml_dtypes').float8_e4m3
    WwT = np.ascontiguousarray(np.asarray(Ww, dtype=f32).T.astype(f8np))
    # [1024, 64] -> [128, 8*64] chunk-packed: row p, col (j, n)
    WgT_p = np.ascontiguousarray(
        np.asarray(Wg, dtype=f32).T.reshape(DCH, 128, NR).transpose(1, 0, 2)
        .reshape(128, DCH * NR).astype(bf16)
    )
    smT_p = np.ascontiguousarray(
        np.asarray(summaries, dtype=f32).T.reshape(DCH, 128, NR).transpose(1, 0, 2)
        .reshape(128, DCH * NR).astype(bf16)
    )
    bc_r = np.ascontiguousarray(np.asarray(bc, dtype=f32).reshape(1, D))
    bg_r = np.ascontiguousarray(np.asarray(bg, dtype=f32).reshape(1, NR))
    bw_r = np.ascontiguousarray(np.asarray(bw, dtype=f32).reshape(1, NN))
    adj_r = np.ascontiguousarray(np.asarray(adjacency, dtype=f32).reshape(1, NN))

    in_maps = []
    for c in range(N_CORES):
        in_maps.append(
            {
                "xs": np.ascontiguousarray(x[c * BSH : (c + 1) * BSH]),
                "WcT": WcT,
                "bc": bc_r,
                "WgT": WgT_p,
                "bg": bg_r,
                "WwT": WwT,
                "bw": bw_r,
                "adjf": adj_r,
                "smT": smT_p,
            }
        )
    return in_maps


def run_kernel_raw(trace=False, **inputs):
    """Returns (gates [32, 64], BassKernelResults)."""
    from concourse.bass_utils import run_bass_kernel_spmd

    nc = _get_nc()
    in_maps = _make_in_maps(**inputs)
    res = run_bass_kernel_spmd(nc, in_maps, list(range(N_CORES)), trace=trace)
    gates = np.concatenate(
        [np.asarray(res.results[c]["gates"]) for c in range(N_CORES)], axis=0
    ).astype(np.float32)
    return gates, res


def kernel(**inputs):
    gates, _ = run_kernel_raw(trace=False, **inputs)
    return gates
